# revision 15
# baseline (speedup 1.0000x reference)
"""Causal multi-head attention on 8 Trainium2 NeuronCores.

Problem: B=2, S=2048, H=1024, NH=16, HD=64, fp32 in/out.
Sharding: tensor-parallel over heads (2 heads/core) + AllToAll so every core
computes the output projection for its own 512-token slice.

Key layout decisions (vs the fp32r baseline):
  * All transposes happen on the HOST: x arrives as xT [H, T] bf16, weights
    arrive pre-transposed bf16 (wqT/wkT/wvT [H, 128], woT [H, H] with rows
    permuted to the AllToAll arrival order). No PE transposes at all.
  * bf16 datapath (PSUM accumulates fp32): removes the fp32r narrow-matmul
    penalty, halves DVE elementwise cost and AllToAll payload.
  * Causal mask is ADDITIVE, applied on the PE into the score PSUM
    (identity x (-1e9 * strict-lower-triangle)), so the per-tile chain is
    PE -> ACT(exp) -> PE with no DVE hop.
  * exp of two adjacent full k-tiles is merged into one ACT instruction over
    a 2-bank PSUM tile ([128,1024]) to amortize ACT's ~222-cycle access cost.
  * QKV biases ride along the PSUM->SBUF copy on DVE (tensor_scalar_add with
    a per-partition scalar); V's bias (free-dim) is a K=1 rank-1 matmul.
  * Output projection is computed transposed (out^T[o,t]) so bo is a
    per-partition scalar; output ships bf16, host casts + un-transposes.

Schedule per core: L1 = per-chunk QKV + head-0 attention (PE-bound, ~95%
busy); AllToAll#0 overlaps L2 = head-1 attention (ACT-bound); E_a (the
head-0 half of the output projection, ct-major) fills the AllToAll#1
window in 8 held PSUM banks; E_b finishes as each ctxa1 tile lands.
Each attention's normalization closure and the NEXT attention's first
score-pair are emitted inside the current attention so the in-order PE
stream never stalls on ACT/DVE.
"""
import sys

if '/opt/trn_rl_repo' not in sys.path:
    sys.path.insert(0, '/opt/trn_rl_repo')

import numpy as np

import concourse.bacc as bacc
import concourse.bass as bass
import concourse.mybir as mybir
from concourse.tile import TileContext
from concourse.bass_utils import run_bass_kernel_spmd
from concourse.masks import make_identity, make_lower_triangular

F32 = mybir.dt.float32
BF16 = mybir.dt.bfloat16
EXP = mybir.ActivationFunctionType.Exp
IDENT = mybir.ActivationFunctionType.Identity

B, S, H, NH, HD = 2, 2048, 1024, 16, 64
NC = 8
T = B * S                 # 4096 tokens
TC = 512                  # tokens per chunk
NCHUNK = T // TC          # 8
NTT = T // 128            # 32 token tiles
HT = H // 128             # 8 H-tiles
SCALE = 1.0 / np.sqrt(HD)
NEG = -1e9
AHEAD = 1

_cache = {}


class _Att:
    """Head-h causal attention for token chunk ch.

    k-tiles are processed in PAIRS sharing one 2-bank PSUM tile so full pairs
    need a single exp instruction. Causal masking is additive on the PE.
    `emit_s()` can be called early (by the PREVIOUS attention, as PE filler)
    to bridge the chunk-boundary exp-wait gap; the normalization closure is
    appended to `deferred` and emitted by the NEXT attention's body.
    """

    def __init__(self, env, ch, h, a2a_in, use_pb):
        self.env = env
        self.ch, self.h, self.a2a_in, self.use_pb = ch, h, a2a_in, use_pb
        self.b, self.lc = ch // 4, ch % 4
        self.nkt = 4 * self.lc + 4
        self.npair = self.nkt // 2
        self.emitted = 0
        self.stps = {}
        self.ctx_ps = None

    def col0(self, kt):
        s = kt - 4 * self.lc
        return 128 * s if s > 0 else 0

    def emit_s(self):
        env, ch, h = self.env, self.ch, self.h
        nc, qpool = env['nc'], env['qpool']
        kT, qT = env['kT'], env['qT']
        j = self.emitted
        stp = qpool.tile([128, 1024], F32, tag='stp', bufs=2, name='stp')
        for i in (0, 1):
            kt = 2 * j + i
            g = 16 * self.b + kt
            s = kt - 4 * self.lc
            c0 = self.col0(kt)
            nc.tensor.matmul(
                stp[:, 512 * i + c0:512 * (i + 1)],
                kT[64 * h:64 * (h + 1), 128 * g:128 * (g + 1)],
                qT[64 * h:64 * (h + 1), TC * ch + c0:TC * (ch + 1)],
                start=True, stop=(s < 0))
            if s >= 0:
                # additive -1e9 strict-lower-triangle on the diagonal block
                nc.tensor.matmul(
                    stp[:, 512 * i + c0:512 * i + c0 + 128],
                    env['ident'][:], env['ltm'][:], start=False, stop=True)
        self.stps[j] = stp
        self.emitted += 1

    def body(self, next_att=None):
        env, ch, h = self.env, self.ch, self.h
        nc, qpool, pc = env['nc'], env['qpool'], env['sc']
        v1, deferred = env['v1'], env['deferred']
        self.ctx_ps = qpool.tile([128, 512], F32, tag='ctx', bufs=2,
                                 name='ctx')
        while self.emitted < min(AHEAD + 1, self.npair):
            self.emit_s()
        for j in range(self.npair):
            stp = self.stps.pop(j)
            kt0 = 2 * j
            diag = (kt0 - 4 * self.lc) >= 0
            p = pc.tile([128, 1024], BF16, tag='p', bufs=3, name='p')
            if not diag:
                nc.scalar.activation(p[:], stp[:], EXP, scale=float(SCALE))
            else:
                for i in (0, 1):
                    c0 = self.col0(kt0 + i)
                    nc.scalar.activation(p[:, 512 * i + c0:512 * (i + 1)],
                                         stp[:, 512 * i + c0:512 * (i + 1)],
                                         EXP, scale=float(SCALE))
            if j == 1 or self.npair == 1:
                for fn in deferred:
                    fn()
                deferred.clear()
            if self.emitted < self.npair:
                self.emit_s()
            elif next_att is not None and next_att.emitted < 2:
                next_att.emit_s()
            for i in (0, 1):
                kt = kt0 + i
                g = 16 * self.b + kt
                c0 = self.col0(kt)
                nc.tensor.matmul(
                    self.ctx_ps[0:65, c0:512],
                    v1[:, 130 * g + 65 * h:130 * g + 65 * h + 65],
                    p[:, 512 * i + c0:512 * (i + 1)],
                    start=(kt == 0), stop=(kt == self.nkt - 1))
        deferred.append(self.norm)

    def norm(self):
        env, ch = self.env, self.ch
        nc, qpool, pc = env['nc'], env['qpool'], env['sc']
        ctx_ps = self.ctx_ps
        rec = pc.tile([1, 512], BF16, tag='rec', bufs=2, name='rec')
        with nc.allow_low_precision(reason='softmax denom recip, ~0.4%'):
            nc.vector.reciprocal(rec[:], ctx_ps[64:65, :])
        bc_sb = pc.tile([64, 512], BF16, tag='bc_sb', bufs=2, name='bc_sb')
        if self.use_pb:
            # GPSIMD broadcast — only while no collective occupies Pool
            nc.gpsimd.partition_broadcast(bc_sb[:], rec[:])
        else:
            bc = qpool.tile([128, 512], F32, tag='proj', bufs=2, name='bc')
            nc.tensor.matmul(bc[0:64, :], env['ones_bf'][0:1, 0:64], rec[:],
                             start=True, stop=True)
            nc.vector.tensor_copy(bc_sb[:], bc[0:64, :])
        ctx_sb = pc.tile([64, 512], BF16, tag='ctx_sb', bufs=2, name='ctx_sb')
        nc.vector.tensor_mul(ctx_sb[:], ctx_ps[0:64, :], bc_sb[:])
        nc.sync.dma_start(self.a2a_in[ch, :, :], ctx_sb[:])


def _build(phases='LE'):
    key = ('nc', phases)
    if key in _cache:
        return _cache[key]
    nc = bacc.Bacc('TRN2', target_bir_lowering=False, debug=False, num_devices=NC)

    xt_d = nc.dram_tensor('xt', [H, T], BF16, kind='ExternalInput')
    wqt_d = nc.dram_tensor('wqt', [H, 128], BF16, kind='ExternalInput')
    wkt_d = nc.dram_tensor('wkt', [H, 128], BF16, kind='ExternalInput')
    wvt_d = nc.dram_tensor('wvt', [H, 128], BF16, kind='ExternalInput')
    wot_d = nc.dram_tensor('wot', [H, H], BF16, kind='ExternalInput')
    bq_d = nc.dram_tensor('bq', [128, 1], F32, kind='ExternalInput')
    bk_d = nc.dram_tensor('bk', [128, 1], F32, kind='ExternalInput')
    bv_d = nc.dram_tensor('bv', [1, 128], F32, kind='ExternalInput')
    bo_d = nc.dram_tensor('bo', [128, HT], F32, kind='ExternalInput')
    out_d = nc.dram_tensor('out', [H, TC], BF16, kind='ExternalOutput')

    with TileContext(nc) as tc:
        with tc.tile_pool(name='persist', bufs=1) as pp, \
             tc.tile_pool(name='scr', bufs=1) as sc, \
             tc.tile_pool(name='dram', bufs=1, space='DRAM') as dpool, \
             tc.tile_pool(name='psum', bufs=1, space='PSUM') as qpool:

            def ptile(shape, dt, tag):
                return pp.tile(shape, dt, tag=tag, name=tag)

            ident = ptile([128, 128], BF16, 'ident')
            make_identity(nc, ident[:])
            ltm = ptile([128, 128], BF16, 'ltm')
            make_lower_triangular(nc, ltm[:], val=NEG, diag=False)
            ones_bf = ptile([1, 128], BF16, 'ones_bf')
            nc.vector.memset(ones_bf[:], 1.0)

            bq_r = ptile([128, 1], F32, 'bq_r')
            bk_r = ptile([128, 1], F32, 'bk_r')
            bo_r = ptile([128, HT], F32, 'bo_r')
            bv_f = ptile([1, 128], F32, 'bv_f')
            bv_bf = ptile([1, 128], BF16, 'bv_bf')

            wqT = ptile([128, HT * 128], BF16, 'wqT')
            wkT = ptile([128, HT * 128], BF16, 'wkT')
            wvT = ptile([128, HT * 128], BF16, 'wvT')
            woT = ptile([128, HT * H], BF16, 'woT')
            xT = ptile([128, HT * T], BF16, 'xT')
            qT = ptile([128, T], BF16, 'qT')
            kT = ptile([128, T], BF16, 'kT')
            v1 = ptile([128, NTT * 130], BF16, 'v1')
            a2a_in0 = dpool.tile([NCHUNK, 64, TC], BF16)
            a2a_out0 = dpool.tile([NCHUNK, 64, TC], BF16)
            a2a_in1 = dpool.tile([NCHUNK, 64, TC], BF16)
            a2a_out1 = dpool.tile([NCHUNK, 64, TC], BF16)

            # ---- input loads, in consumption order (DMA device is serial):
            # biases (tiny), x chunk 0 per-h-tile (so QKV(0) streams), q/k/v
            # weights, remaining x chunks. woT/bo only feed E — loaded
            # mid-L1 (emitted after X0's collective).
            xt_r = xt_d[:].rearrange('(ht p) t -> p ht t', p=128)
            xT_r = xT[:].rearrange('p (ht t) -> p ht t', ht=HT)

            for dst, src in ((bq_r, bq_d), (bk_r, bk_d), (bv_f, bv_d)):
                nc.sync.dma_start(dst[:], src[:])
            nc.vector.tensor_copy(bv_bf[:], bv_f[:])
            for ht in range(HT):
                nc.sync.dma_start(xT_r[:, ht, 0:TC], xt_r[:, ht, 0:TC])
            for w_sb, w_d in ((wqT, wqt_d), (wkT, wkt_d), (wvT, wvt_d)):
                nc.sync.dma_start(
                    w_sb[:].rearrange('p (ht c) -> p ht c', ht=HT),
                    w_d[:].rearrange('(ht p) c -> p ht c', p=128))
            for ch in range(1, NCHUNK):
                nc.sync.dma_start(xT_r[:, :, TC * ch:TC * (ch + 1)],
                                  xt_r[:, :, TC * ch:TC * (ch + 1)])

            # v1 ones columns (col 64 of each 65-block)
            ones_dst = bass.AP(v1.tensor, v1.offset + 64,
                               [list(v1.ap[0]), [130, NTT], [65, 2]])
            nc.vector.memset(ones_dst, 1.0)

            deferred = []
            env = dict(nc=nc, qpool=qpool, sc=sc, qT=qT, kT=kT, v1=v1,
                       ident=ident, ltm=ltm, ones_bf=ones_bf,
                       deferred=deferred)

            def qkv(ch):
                for w_t, b_t, dst in ((wqT, bq_r, qT), (wkT, bk_r, kT)):
                    ps = qpool.tile([128, 512], F32, tag='proj', bufs=2,
                                    name='proj')
                    for ht in range(HT):
                        nc.tensor.matmul(
                            ps[:], w_t[:, 128 * ht:128 * (ht + 1)],
                            xT_r[:, ht, TC * ch:TC * (ch + 1)],
                            start=(ht == 0), stop=(ht == HT - 1))
                    nc.vector.tensor_scalar_add(
                        dst[:, TC * ch:TC * (ch + 1)], ps[:], b_t[:])
                # V in token-major: out[t, c], 4 regions of one bank
                ps = qpool.tile([128, 512], F32, tag='proj', bufs=2,
                                name='proj')
                for tt in range(4):
                    r = ps[:, 128 * tt:128 * (tt + 1)]
                    for ht in range(HT):
                        nc.tensor.matmul(
                            r,
                            xT_r[:, ht,
                                 TC * ch + 128 * tt:TC * ch + 128 * (tt + 1)],
                            wvT[:, 128 * ht:128 * (ht + 1)],
                            start=(ht == 0), stop=False)
                    nc.tensor.matmul(r, ones_bf[:], bv_bf[:],
                                     start=False, stop=True)
                    kt = 4 * ch + tt
                    base = 130 * kt
                    # [V_h0 | gap | V_h1]: strided copy fills cols
                    # base..base+63 and base+65..base+128
                    dst = bass.AP(v1.tensor, v1.offset + base,
                                  [list(v1.ap[0]), [65, 2], [1, 64]])
                    nc.vector.tensor_copy(
                        dst,
                        ps[:, 128 * tt:128 * (tt + 1)].rearrange(
                            'p (g c) -> p g c', g=2))

            ctxa = []
            if 'L' in phases:
                l1 = [_Att(env, ch, 0, a2a_in0, True) for ch in range(NCHUNK)]
                l2 = [_Att(env, ch, 1, a2a_in1, False) for ch in range(NCHUNK)]
                # ---- L1: per-chunk QKV + head-0 attention ----
                for ch in range(NCHUNK):
                    qkv(ch)
                    # next chunk's QKV is the natural PE filler here; a
                    # starter would read qT/kT before they're written
                    l1[ch].body(l2[0] if ch + 1 == NCHUNK else None)
                for fn in deferred:
                    fn()
                deferred.clear()
                # ---- X0: AllToAll for head 0 (overlaps L2) ----
                nc.gpsimd.collective_compute(
                    'AllToAll', mybir.AluOpType.bypass,
                    replica_groups=[list(range(NC))],
                    ins=[a2a_in0[:]], outs=[a2a_out0[:]],
                )
                # E-phase weights + head-0 ctx (lands mid-L2): load now so
                # E_a can fill the AllToAll#1 window (SP queue is in-order)
                nc.sync.dma_start(
                    woT[:].rearrange('p (ct o) -> p ct o', ct=HT),
                    wot_d[:].rearrange('(ct p) o -> p ct o', p=128))
                nc.sync.dma_start(bo_r[:], bo_d[:])
                for jj in range(4):
                    t = ptile([128, 512], BF16, f'ctxa0{jj}')
                    nc.sync.dma_start(
                        t[:],
                        a2a_out0[2 * jj:2 * jj + 2, :, :].rearrange(
                            'a b t -> (a b) t'))
                    ctxa.append(t)

                # ---- L2: head-1 attention ----
                for ch in range(NCHUNK):
                    l2[ch].body(l2[ch + 1] if ch + 1 < NCHUNK else None)
                for fn in deferred:
                    fn()
                deferred.clear()
                nc.gpsimd.collective_compute(
                    'AllToAll', mybir.AluOpType.bypass,
                    replica_groups=[list(range(NC))],
                    ins=[a2a_in1[:]], outs=[a2a_out1[:]],
                )

            # ---- E: transposed output projection for my 512 tokens ----
            # ct-major: E_a (head-0 contraction half) runs during AllToAll#1
            # in 8 held PSUM banks; E_b consumes each ctxa1 tile as it lands;
            # the final ct walks ot-major so copies/stores pipeline out.
            if 'E' in phases:
                slots = []
                for _ in range(2):
                    tl = qpool.tile([128, 1024], F32, tag='stp', bufs=2,
                                    name='stp')
                    slots += [(tl, 0), (tl, 512)]
                for tag in ('proj', 'proj', 'ctx', 'ctx'):
                    slots.append(
                        (qpool.tile([128, 512], F32, tag=tag, bufs=2,
                                    name=tag), 0))

                def e_pass(ct, start, stop, drain=False):
                    for ot in range(HT):
                        tl, c0 = slots[ot]
                        reg = tl[:, c0:c0 + 512]
                        nc.tensor.matmul(
                            reg,
                            woT[:, H * ct + 128 * ot:H * ct + 128 * (ot + 1)],
                            ctxa[ct][:],
                            start=start, stop=stop)
                        if drain:
                            o_sb = sc.tile([128, 512], BF16, tag='o_sb',
                                           bufs=4, name='o_sb')
                            if ot % 2 == 0:
                                nc.vector.tensor_scalar_add(
                                    o_sb[:], reg, bo_r[:, ot:ot + 1])
                            else:
                                nc.scalar.activation(
                                    o_sb[:], reg, IDENT,
                                    bias=bo_r[:, ot:ot + 1])
                            nc.sync.dma_start(
                                out_d[128 * ot:128 * (ot + 1), :], o_sb[:])

                for ct in range(4):
                    e_pass(ct, ct == 0, False)
                for jj in range(4):
                    t = ptile([128, 512], BF16, f'ctxa1{jj}')
                    nc.sync.dma_start(
                        t[:],
                        a2a_out1[2 * jj:2 * jj + 2, :, :].rearrange(
                            'a b t -> (a b) t'))
                    ctxa.append(t)
                for ct in range(4, HT - 1):
                    e_pass(ct, False, False)
                e_pass(HT - 1, False, True, drain=True)

    nc.compile()
    _cache[key] = nc
    return nc


def kernel(hidden_states, Wq, bq, Wk, bk, Wv, bv, Wo, bo, **run_kwargs):
    import ml_dtypes
    bf16 = ml_dtypes.bfloat16
    nc = _build()
    hs = np.asarray(hidden_states, np.float32).reshape(T, H)
    xT = np.ascontiguousarray(hs.T.astype(bf16))
    Wq, Wk, Wv, Wo = (np.asarray(w, np.float32) for w in (Wq, Wk, Wv, Wo))
    bq, bk, bv, bo = (np.asarray(b, np.float32) for b in (bq, bk, bv, bo))
    # woT rows permuted to AllToAll arrival order: [p0h0, p1h0, .., p7h0,
    # p0h1, ..], where peer p's head h covers channels 128p+64h .. +64.
    perm = np.concatenate([np.arange(128 * p + 64 * h, 128 * p + 64 * h + 64)
                           for h in range(2) for p in range(NC)])
    woT = np.ascontiguousarray(Wo.T[perm].astype(bf16))
    bo_m = np.ascontiguousarray(bo.reshape(HT, 128).T)
    in_maps = []
    for c in range(NC):
        r = slice(128 * c, 128 * (c + 1))
        in_maps.append({
            'xt': xT,
            'wqt': np.ascontiguousarray(Wq[r].T.astype(bf16)),
            'wkt': np.ascontiguousarray(Wk[r].T.astype(bf16)),
            'wvt': np.ascontiguousarray(Wv[r].T.astype(bf16)),
            'wot': woT,
            'bq': np.ascontiguousarray(bq[r].reshape(128, 1)),
            'bk': np.ascontiguousarray(bk[r].reshape(128, 1)),
            'bv': np.ascontiguousarray(bv[r].reshape(1, 128)),
            'bo': bo_m,
        })
    res = run_bass_kernel_spmd(nc, in_maps, core_ids=list(range(NC)), **run_kwargs)
    out = np.empty((T, H), np.float32)
    for c in range(NC):
        out[TC * c:TC * (c + 1), :] = res.results[c]['out'].astype(np.float32).T
    kernel.last_results = res
    return out.reshape(B, S, H)


# revision 17
# speedup vs baseline: 1.0272x; 1.0272x over previous
"""Causal multi-head attention on 8 Trainium2 NeuronCores.

Problem: B=2, S=2048, H=1024, NH=16, HD=64, fp32 in/out.
Sharding: tensor-parallel over heads (2 heads/core) + AllToAll so every core
computes the output projection for its own 512-token slice.

Key layout decisions (vs the fp32r baseline):
  * All transposes happen on the HOST: x arrives as xT [H, T] bf16, weights
    arrive pre-transposed bf16 (wqT/wkT/wvT [H, 128], woT [H, H] with rows
    permuted to the AllToAll arrival order). No PE transposes at all.
  * bf16 datapath (PSUM accumulates fp32): removes the fp32r narrow-matmul
    penalty, halves DVE elementwise cost and AllToAll payload.
  * Causal mask is ADDITIVE, applied on the PE into the score PSUM
    (identity x (-1e9 * strict-lower-triangle)), so the per-tile chain is
    PE -> ACT(exp) -> PE with no DVE hop.
  * exp of two adjacent full k-tiles is merged into one ACT instruction over
    a 2-bank PSUM tile ([128,1024]) to amortize ACT's ~222-cycle access cost.
  * QKV biases ride along the PSUM->SBUF copy on DVE (tensor_scalar_add with
    a per-partition scalar); V's bias (free-dim) is a K=1 rank-1 matmul.
  * Output projection is computed transposed (out^T[o,t]) so bo is a
    per-partition scalar; output ships bf16, host casts + un-transposes.

Schedule per core: L1 = per-chunk QKV + head-0 attention (PE-bound, ~95%
busy); AllToAll#0 overlaps L2 = head-1 attention (ACT-bound); E_a (the
head-0 half of the output projection, ct-major) fills the AllToAll#1
window in 8 held PSUM banks; E_b finishes as each ctxa1 tile lands.
Each attention's normalization closure and the NEXT attention's first
score-pair are emitted inside the current attention so the in-order PE
stream never stalls on ACT/DVE.
"""
import sys

if '/opt/trn_rl_repo' not in sys.path:
    sys.path.insert(0, '/opt/trn_rl_repo')

import numpy as np

import concourse.bacc as bacc
import concourse.bass as bass
import concourse.mybir as mybir
from concourse.tile import TileContext
from concourse.bass_utils import run_bass_kernel_spmd
from concourse.masks import make_identity, make_lower_triangular

F32 = mybir.dt.float32
BF16 = mybir.dt.bfloat16
EXP = mybir.ActivationFunctionType.Exp
IDENT = mybir.ActivationFunctionType.Identity

B, S, H, NH, HD = 2, 2048, 1024, 16, 64
NC = 8
T = B * S                 # 4096 tokens
TC = 512                  # tokens per chunk
NCHUNK = T // TC          # 8
NTT = T // 128            # 32 token tiles
HT = H // 128             # 8 H-tiles
SCALE = 1.0 / np.sqrt(HD)
NEG = -1e9
AHEAD = 1

_cache = {}


class _Att:
    """Head-h causal attention for token chunk ch.

    k-tiles are processed in PAIRS sharing one 2-bank PSUM tile so full pairs
    need a single exp instruction. Causal masking is additive on the PE.
    `emit_s()` can be called early (by the PREVIOUS attention, as PE filler)
    to bridge the chunk-boundary exp-wait gap; the normalization closure is
    appended to `deferred` and emitted by the NEXT attention's body.
    """

    def __init__(self, env, ch, h, a2a_in, use_pb):
        self.env = env
        self.ch, self.h, self.a2a_in, self.use_pb = ch, h, a2a_in, use_pb
        self.b, self.lc = ch // 4, ch % 4
        self.nkt = 4 * self.lc + 4
        self.npair = self.nkt // 2
        self.emitted = 0
        self.stps = {}
        self.ctx_ps = None

    def col0(self, kt):
        s = kt - 4 * self.lc
        return 128 * s if s > 0 else 0

    def emit_s(self):
        env, ch, h = self.env, self.ch, self.h
        nc, qpool = env['nc'], env['qpool']
        kT, qT = env['kT'], env['qT']
        j = self.emitted
        stp = qpool.tile([128, 1024], F32, tag='stp', bufs=2, name='stp')
        for i in (0, 1):
            kt = 2 * j + i
            g = 16 * self.b + kt
            s = kt - 4 * self.lc
            c0 = self.col0(kt)
            nc.tensor.matmul(
                stp[:, 512 * i + c0:512 * (i + 1)],
                kT[64 * h:64 * (h + 1), 128 * g:128 * (g + 1)],
                qT[64 * h:64 * (h + 1), TC * ch + c0:TC * (ch + 1)],
                start=True, stop=(s < 0))
            if s >= 0:
                # additive -1e9 strict-lower-triangle on the diagonal block
                nc.tensor.matmul(
                    stp[:, 512 * i + c0:512 * i + c0 + 128],
                    env['ident'][:], env['ltm'][:], start=False, stop=True)
        self.stps[j] = stp
        self.emitted += 1

    def body(self, next_att=None):
        env, ch, h = self.env, self.ch, self.h
        nc, qpool, pc = env['nc'], env['qpool'], env['sc']
        v1, deferred = env['v1'], env['deferred']
        self.ctx_ps = qpool.tile([128, 512], F32, tag='ctx', bufs=2,
                                 name='ctx')
        while self.emitted < min(AHEAD + 1, self.npair):
            self.emit_s()
        for j in range(self.npair):
            stp = self.stps.pop(j)
            kt0 = 2 * j
            diag = (kt0 - 4 * self.lc) >= 0
            p = pc.tile([128, 1024], BF16, tag='p', bufs=3, name='p')
            if not diag:
                nc.scalar.activation(p[:], stp[:], EXP, scale=float(SCALE))
            else:
                for i in (0, 1):
                    c0 = self.col0(kt0 + i)
                    nc.scalar.activation(p[:, 512 * i + c0:512 * (i + 1)],
                                         stp[:, 512 * i + c0:512 * (i + 1)],
                                         EXP, scale=float(SCALE))
            if j == 1 or self.npair == 1:
                for fn in deferred:
                    fn()
                deferred.clear()
            if self.emitted < self.npair:
                self.emit_s()
            elif next_att is not None and next_att.emitted < 1:
                next_att.emit_s()
            for i in (0, 1):
                kt = kt0 + i
                g = 16 * self.b + kt
                c0 = self.col0(kt)
                nc.tensor.matmul(
                    self.ctx_ps[0:65, c0:512],
                    v1[:, 130 * g + 65 * h:130 * g + 65 * h + 65],
                    p[:, 512 * i + c0:512 * (i + 1)],
                    start=(kt == 0), stop=(kt == self.nkt - 1))
        deferred.append(self.norm)

    def norm(self):
        env, ch = self.env, self.ch
        nc, qpool, pc = env['nc'], env['qpool'], env['sc']
        ctx_ps = self.ctx_ps
        rec = pc.tile([1, 512], BF16, tag='rec', bufs=2, name='rec')
        with nc.allow_low_precision(reason='softmax denom recip, ~0.4%'):
            nc.vector.reciprocal(rec[:], ctx_ps[64:65, :])
        bc_sb = pc.tile([64, 512], BF16, tag='bc_sb', bufs=2, name='bc_sb')
        if self.use_pb:
            # GPSIMD broadcast — only while no collective occupies Pool
            nc.gpsimd.partition_broadcast(bc_sb[:], rec[:])
        else:
            bc = qpool.tile([128, 512], F32, tag='proj', bufs=2, name='bc')
            nc.tensor.matmul(bc[0:64, :], env['ones_bf'][0:1, 0:64], rec[:],
                             start=True, stop=True)
            nc.vector.tensor_copy(bc_sb[:], bc[0:64, :])
        ctx_sb = pc.tile([64, 512], BF16, tag='ctx_sb', bufs=2, name='ctx_sb')
        nc.vector.tensor_mul(ctx_sb[:], ctx_ps[0:64, :], bc_sb[:])
        nc.sync.dma_start(self.a2a_in[ch, :, :], ctx_sb[:])


def _build(phases='LE'):
    key = ('nc', phases)
    if key in _cache:
        return _cache[key]
    nc = bacc.Bacc('TRN2', target_bir_lowering=False, debug=False, num_devices=NC)

    xt_d = nc.dram_tensor('xt', [H, T], BF16, kind='ExternalInput')
    wqt_d = nc.dram_tensor('wqt', [H, 128], BF16, kind='ExternalInput')
    wkt_d = nc.dram_tensor('wkt', [H, 128], BF16, kind='ExternalInput')
    wvt_d = nc.dram_tensor('wvt', [H, 128], BF16, kind='ExternalInput')
    wot_d = nc.dram_tensor('wot', [H, H], BF16, kind='ExternalInput')
    bq_d = nc.dram_tensor('bq', [128, 1], F32, kind='ExternalInput')
    bk_d = nc.dram_tensor('bk', [128, 1], F32, kind='ExternalInput')
    bv_d = nc.dram_tensor('bv', [1, 128], F32, kind='ExternalInput')
    bo_d = nc.dram_tensor('bo', [128, HT], F32, kind='ExternalInput')
    out_d = nc.dram_tensor('out', [H, TC], BF16, kind='ExternalOutput')

    with TileContext(nc) as tc:
        with tc.tile_pool(name='persist', bufs=1) as pp, \
             tc.tile_pool(name='scr', bufs=1) as sc, \
             tc.tile_pool(name='dram', bufs=1, space='DRAM') as dpool, \
             tc.tile_pool(name='psum', bufs=1, space='PSUM') as qpool:

            def ptile(shape, dt, tag):
                return pp.tile(shape, dt, tag=tag, name=tag)

            ident = ptile([128, 128], BF16, 'ident')
            make_identity(nc, ident[:])
            ltm = ptile([128, 128], BF16, 'ltm')
            make_lower_triangular(nc, ltm[:], val=NEG, diag=False)
            ones_bf = ptile([1, 128], BF16, 'ones_bf')
            nc.vector.memset(ones_bf[:], 1.0)

            bq_r = ptile([128, 1], F32, 'bq_r')
            bk_r = ptile([128, 1], F32, 'bk_r')
            bo_r = ptile([128, HT], F32, 'bo_r')
            bv_f = ptile([1, 128], F32, 'bv_f')
            bv_bf = ptile([1, 128], BF16, 'bv_bf')

            wqT = ptile([128, HT * 128], BF16, 'wqT')
            wkT = ptile([128, HT * 128], BF16, 'wkT')
            wvT = ptile([128, HT * 128], BF16, 'wvT')
            woT = ptile([128, HT * H], BF16, 'woT')
            xT = ptile([128, HT * T], BF16, 'xT')
            qT = ptile([128, T], BF16, 'qT')
            kT = ptile([128, T], BF16, 'kT')
            v1 = ptile([128, NTT * 130], BF16, 'v1')
            a2a_in0 = dpool.tile([NCHUNK, 64, TC], BF16)
            a2a_out0 = dpool.tile([NCHUNK, 64, TC], BF16)
            a2a_in1 = dpool.tile([NCHUNK, 64, TC], BF16)
            a2a_out1 = dpool.tile([NCHUNK, 64, TC], BF16)

            # ---- input loads, in consumption order (DMA device is serial):
            # biases (tiny), x chunk 0 per-h-tile (so QKV(0) streams), q/k/v
            # weights, remaining x chunks. woT/bo only feed E — loaded
            # mid-L1 (emitted after X0's collective).
            xt_r = xt_d[:].rearrange('(ht p) t -> p ht t', p=128)
            xT_r = xT[:].rearrange('p (ht t) -> p ht t', ht=HT)

            nc.sync.dma_start(
                wqT[:].rearrange('p (ht c) -> p ht c', ht=HT),
                wqt_d[:].rearrange('(ht p) c -> p ht c', p=128))
            for ht in range(HT):
                nc.sync.dma_start(xT_r[:, ht, 0:TC], xt_r[:, ht, 0:TC])
            for w_sb, w_d in ((wkT, wkt_d), (wvT, wvt_d)):
                nc.sync.dma_start(
                    w_sb[:].rearrange('p (ht c) -> p ht c', ht=HT),
                    w_d[:].rearrange('(ht p) c -> p ht c', p=128))
            for dst, src in ((bq_r, bq_d), (bk_r, bk_d), (bv_f, bv_d)):
                nc.sync.dma_start(dst[:], src[:])
            nc.vector.tensor_copy(bv_bf[:], bv_f[:])
            for ch in range(1, NCHUNK):
                nc.sync.dma_start(xT_r[:, :, TC * ch:TC * (ch + 1)],
                                  xt_r[:, :, TC * ch:TC * (ch + 1)])

            # v1 ones columns (col 64 of each 65-block)
            ones_dst = bass.AP(v1.tensor, v1.offset + 64,
                               [list(v1.ap[0]), [130, NTT], [65, 2]])
            nc.vector.memset(ones_dst, 1.0)

            deferred = []
            env = dict(nc=nc, qpool=qpool, sc=sc, qT=qT, kT=kT, v1=v1,
                       ident=ident, ltm=ltm, ones_bf=ones_bf,
                       deferred=deferred)

            def qkv(ch):
                for w_t, b_t, dst in ((wqT, bq_r, qT), (wkT, bk_r, kT)):
                    ps = qpool.tile([128, 512], F32, tag='proj', bufs=2,
                                    name='proj')
                    for ht in range(HT):
                        nc.tensor.matmul(
                            ps[:], w_t[:, 128 * ht:128 * (ht + 1)],
                            xT_r[:, ht, TC * ch:TC * (ch + 1)],
                            start=(ht == 0), stop=(ht == HT - 1))
                    nc.vector.tensor_scalar_add(
                        dst[:, TC * ch:TC * (ch + 1)], ps[:], b_t[:])
                # V in token-major: out[t, c], 4 regions of one bank
                ps = qpool.tile([128, 512], F32, tag='proj', bufs=2,
                                name='proj')
                for tt in range(4):
                    r = ps[:, 128 * tt:128 * (tt + 1)]
                    for ht in range(HT):
                        nc.tensor.matmul(
                            r,
                            xT_r[:, ht,
                                 TC * ch + 128 * tt:TC * ch + 128 * (tt + 1)],
                            wvT[:, 128 * ht:128 * (ht + 1)],
                            start=(ht == 0), stop=False)
                    nc.tensor.matmul(r, ones_bf[:], bv_bf[:],
                                     start=False, stop=True)
                    kt = 4 * ch + tt
                    base = 130 * kt
                    # [V_h0 | gap | V_h1]: strided copy fills cols
                    # base..base+63 and base+65..base+128
                    dst = bass.AP(v1.tensor, v1.offset + base,
                                  [list(v1.ap[0]), [65, 2], [1, 64]])
                    nc.vector.tensor_copy(
                        dst,
                        ps[:, 128 * tt:128 * (tt + 1)].rearrange(
                            'p (g c) -> p g c', g=2))

            ctxa = []
            if 'L' in phases:
                l1 = [_Att(env, ch, 0, a2a_in0, True) for ch in range(NCHUNK)]
                l2 = [_Att(env, ch, 1, a2a_in1, False) for ch in range(NCHUNK)]
                # ---- L1: per-chunk QKV + head-0 attention ----
                for ch in range(NCHUNK):
                    qkv(ch)
                    # next chunk's QKV is the natural PE filler here; a
                    # starter would read qT/kT before they're written
                    l1[ch].body(l2[0] if ch + 1 == NCHUNK else None)
                for fn in deferred:
                    fn()
                deferred.clear()
                # ---- X0: AllToAll for head 0 (overlaps L2) ----
                nc.gpsimd.collective_compute(
                    'AllToAll', mybir.AluOpType.bypass,
                    replica_groups=[list(range(NC))],
                    ins=[a2a_in0[:]], outs=[a2a_out0[:]],
                )
                # E-phase weights + head-0 ctx (lands mid-L2): load now so
                # E_a can fill the AllToAll#1 window (SP queue is in-order)
                nc.sync.dma_start(
                    woT[:].rearrange('p (ct o) -> p ct o', ct=HT),
                    wot_d[:].rearrange('(ct p) o -> p ct o', p=128))
                nc.sync.dma_start(bo_r[:], bo_d[:])
                for jj in range(4):
                    t = ptile([128, 512], BF16, f'ctxa0{jj}')
                    nc.sync.dma_start(
                        t[:],
                        a2a_out0[2 * jj:2 * jj + 2, :, :].rearrange(
                            'a b t -> (a b) t'))
                    ctxa.append(t)

                # ---- L2: head-1 attention ----
                for ch in range(NCHUNK):
                    l2[ch].body(l2[ch + 1] if ch + 1 < NCHUNK else None)
                for fn in deferred:
                    fn()
                deferred.clear()
                nc.gpsimd.collective_compute(
                    'AllToAll', mybir.AluOpType.bypass,
                    replica_groups=[list(range(NC))],
                    ins=[a2a_in1[:]], outs=[a2a_out1[:]],
                )

            # ---- E: transposed output projection for my 512 tokens ----
            # ct-major: E_a (head-0 contraction half) runs during AllToAll#1
            # in 8 held PSUM banks; E_b consumes each ctxa1 tile as it lands;
            # the final ct walks ot-major so copies/stores pipeline out.
            if 'E' in phases:
                slots = []
                for _ in range(2):
                    tl = qpool.tile([128, 1024], F32, tag='stp', bufs=2,
                                    name='stp')
                    slots += [(tl, 0), (tl, 512)]
                for tag in ('proj', 'proj', 'ctx', 'ctx'):
                    slots.append(
                        (qpool.tile([128, 512], F32, tag=tag, bufs=2,
                                    name=tag), 0))

                def e_pass(ct, start, stop, drain=False):
                    for ot in range(HT):
                        tl, c0 = slots[ot]
                        reg = tl[:, c0:c0 + 512]
                        nc.tensor.matmul(
                            reg,
                            woT[:, H * ct + 128 * ot:H * ct + 128 * (ot + 1)],
                            ctxa[ct][:],
                            start=start, stop=stop)
                        if drain:
                            o_sb = sc.tile([128, 512], BF16, tag='o_sb',
                                           bufs=4, name='o_sb')
                            if ot % 2 == 0:
                                nc.vector.tensor_scalar_add(
                                    o_sb[:], reg, bo_r[:, ot:ot + 1])
                            else:
                                nc.scalar.activation(
                                    o_sb[:], reg, IDENT,
                                    bias=bo_r[:, ot:ot + 1])
                            nc.sync.dma_start(
                                out_d[128 * ot:128 * (ot + 1), :], o_sb[:])

                for ct in range(4):
                    e_pass(ct, ct == 0, False)
                for jj in range(4):
                    t = ptile([128, 512], BF16, f'ctxa1{jj}')
                    nc.sync.dma_start(
                        t[:],
                        a2a_out1[2 * jj:2 * jj + 2, :, :].rearrange(
                            'a b t -> (a b) t'))
                    ctxa.append(t)
                for ct in range(4, HT - 1):
                    e_pass(ct, False, False)
                e_pass(HT - 1, False, True, drain=True)

    nc.compile()
    _cache[key] = nc
    return nc


def kernel(hidden_states, Wq, bq, Wk, bk, Wv, bv, Wo, bo, **run_kwargs):
    import ml_dtypes
    bf16 = ml_dtypes.bfloat16
    nc = _build()
    hs = np.asarray(hidden_states, np.float32).reshape(T, H)
    xT = np.ascontiguousarray(hs.T.astype(bf16))
    Wq, Wk, Wv, Wo = (np.asarray(w, np.float32) for w in (Wq, Wk, Wv, Wo))
    bq, bk, bv, bo = (np.asarray(b, np.float32) for b in (bq, bk, bv, bo))
    # woT rows permuted to AllToAll arrival order: [p0h0, p1h0, .., p7h0,
    # p0h1, ..], where peer p's head h covers channels 128p+64h .. +64.
    perm = np.concatenate([np.arange(128 * p + 64 * h, 128 * p + 64 * h + 64)
                           for h in range(2) for p in range(NC)])
    woT = np.ascontiguousarray(Wo.T[perm].astype(bf16))
    bo_m = np.ascontiguousarray(bo.reshape(HT, 128).T)
    in_maps = []
    for c in range(NC):
        r = slice(128 * c, 128 * (c + 1))
        in_maps.append({
            'xt': xT,
            'wqt': np.ascontiguousarray(Wq[r].T.astype(bf16)),
            'wkt': np.ascontiguousarray(Wk[r].T.astype(bf16)),
            'wvt': np.ascontiguousarray(Wv[r].T.astype(bf16)),
            'wot': woT,
            'bq': np.ascontiguousarray(bq[r].reshape(128, 1)),
            'bk': np.ascontiguousarray(bk[r].reshape(128, 1)),
            'bv': np.ascontiguousarray(bv[r].reshape(1, 128)),
            'bo': bo_m,
        })
    res = run_bass_kernel_spmd(nc, in_maps, core_ids=list(range(NC)), **run_kwargs)
    out = np.empty((T, H), np.float32)
    for c in range(NC):
        out[TC * c:TC * (c + 1), :] = res.results[c]['out'].astype(np.float32).T
    kernel.last_results = res
    return out.reshape(B, S, H)


# revision 18
# speedup vs baseline: 1.0556x; 1.0276x over previous
"""Causal multi-head attention on 8 Trainium2 NeuronCores.

Problem: B=2, S=2048, H=1024, NH=16, HD=64, fp32 in/out.
Sharding: tensor-parallel over heads (2 heads/core) + AllToAll so every core
computes the output projection for its own 512-token slice.

Key layout decisions (vs the fp32r baseline):
  * All transposes happen on the HOST: x arrives as xT [H, T] bf16, weights
    arrive pre-transposed bf16 (wqT/wkT/wvT [H, 128], woT [H, H] with rows
    permuted to the AllToAll arrival order). No PE transposes at all.
  * bf16 datapath (PSUM accumulates fp32): removes the fp32r narrow-matmul
    penalty, halves DVE elementwise cost and AllToAll payload.
  * Causal mask is ADDITIVE, applied on the PE into the score PSUM
    (identity x (-1e9 * strict-lower-triangle)), so the per-tile chain is
    PE -> ACT(exp) -> PE with no DVE hop.
  * exp of two adjacent full k-tiles is merged into one ACT instruction over
    a 2-bank PSUM tile ([128,1024]) to amortize ACT's ~222-cycle access cost.
  * QKV biases ride along the PSUM->SBUF copy on DVE (tensor_scalar_add with
    a per-partition scalar); V's bias (free-dim) is a K=1 rank-1 matmul.
  * Output projection is computed transposed (out^T[o,t]) so bo is a
    per-partition scalar; output ships bf16, host casts + un-transposes.

Schedule per core: L1 = per-chunk QKV + head-0 attention (PE-bound, ~95%
busy); AllToAll#0 overlaps L2 = head-1 attention (ACT-bound); E_a (the
head-0 half of the output projection, ct-major) fills the AllToAll#1
window in 8 held PSUM banks; E_b finishes as each ctxa1 tile lands.
Each attention's normalization closure and the NEXT attention's first
score-pair are emitted inside the current attention so the in-order PE
stream never stalls on ACT/DVE.
"""
import sys

if '/opt/trn_rl_repo' not in sys.path:
    sys.path.insert(0, '/opt/trn_rl_repo')

import numpy as np

import concourse.bacc as bacc
import concourse.bass as bass
import concourse.mybir as mybir
from concourse.tile import TileContext
from concourse.bass_utils import run_bass_kernel_spmd
from concourse.masks import make_identity, make_lower_triangular

F32 = mybir.dt.float32
BF16 = mybir.dt.bfloat16
EXP = mybir.ActivationFunctionType.Exp
IDENT = mybir.ActivationFunctionType.Identity

B, S, H, NH, HD = 2, 2048, 1024, 16, 64
NC = 8
T = B * S                 # 4096 tokens
TC = 512                  # tokens per chunk
NCHUNK = T // TC          # 8
NTT = T // 128            # 32 token tiles
HT = H // 128             # 8 H-tiles
SCALE = 1.0 / np.sqrt(HD)
NEG = -1e9
AHEAD = 1

_cache = {}


class _Att:
    """Head-h causal attention for token chunk ch.

    k-tiles are processed in PAIRS sharing one 2-bank PSUM tile so full pairs
    need a single exp instruction. Causal masking is additive on the PE.
    `emit_s()` can be called early (by the PREVIOUS attention, as PE filler)
    to bridge the chunk-boundary exp-wait gap; the normalization closure is
    appended to `deferred` and emitted by the NEXT attention's body.
    """

    def __init__(self, env, ch, h, a2a_in, use_pb):
        self.env = env
        self.ch, self.h, self.a2a_in, self.use_pb = ch, h, a2a_in, use_pb
        self.b, self.lc = ch // 4, ch % 4
        self.nkt = 4 * self.lc + 4
        self.npair = self.nkt // 2
        self.emitted = 0
        self.stps = {}
        self.ctx_ps = None

    def col0(self, kt):
        s = kt - 4 * self.lc
        return 128 * s if s > 0 else 0

    def emit_s(self):
        env, ch, h = self.env, self.ch, self.h
        nc, qpool = env['nc'], env['qpool']
        kT, qT = env['kT'], env['qT']
        j = self.emitted
        stp = qpool.tile([128, 1024], F32, tag='stp', bufs=2, name='stp')
        for i in (0, 1):
            kt = 2 * j + i
            g = 16 * self.b + kt
            s = kt - 4 * self.lc
            c0 = self.col0(kt)
            nc.tensor.matmul(
                stp[:, 512 * i + c0:512 * (i + 1)],
                kT[64 * h:64 * (h + 1), 128 * g:128 * (g + 1)],
                qT[64 * h:64 * (h + 1), TC * ch + c0:TC * (ch + 1)],
                start=True, stop=(s < 0))
            if s >= 0:
                # additive -1e9 strict-lower-triangle on the diagonal block
                nc.tensor.matmul(
                    stp[:, 512 * i + c0:512 * i + c0 + 128],
                    env['ident'][:], env['ltm'][:], start=False, stop=True)
        self.stps[j] = stp
        self.emitted += 1

    def body(self, next_att=None):
        env, ch, h = self.env, self.ch, self.h
        nc, qpool, pc = env['nc'], env['qpool'], env['sc']
        v1, deferred = env['v1'], env['deferred']
        self.ctx_ps = qpool.tile([128, 512], F32, tag='ctx', bufs=2,
                                 name='ctx')
        while self.emitted < min(AHEAD + 1, self.npair):
            self.emit_s()
        for j in range(self.npair):
            stp = self.stps.pop(j)
            kt0 = 2 * j
            diag = (kt0 - 4 * self.lc) >= 0
            p = pc.tile([128, 1024], BF16, tag='p', bufs=3, name='p')
            if not diag:
                nc.scalar.activation(p[:], stp[:], EXP, scale=float(SCALE))
            else:
                for i in (0, 1):
                    c0 = self.col0(kt0 + i)
                    nc.scalar.activation(p[:, 512 * i + c0:512 * (i + 1)],
                                         stp[:, 512 * i + c0:512 * (i + 1)],
                                         EXP, scale=float(SCALE))
            if j == 1 or self.npair == 1:
                for fn in deferred:
                    fn()
                deferred.clear()
            if self.emitted < self.npair:
                self.emit_s()
            elif next_att is not None and next_att.emitted < 0:
                next_att.emit_s()
            for i in (0, 1):
                kt = kt0 + i
                g = 16 * self.b + kt
                c0 = self.col0(kt)
                nc.tensor.matmul(
                    self.ctx_ps[0:65, c0:512],
                    v1[:, 130 * g + 65 * h:130 * g + 65 * h + 65],
                    p[:, 512 * i + c0:512 * (i + 1)],
                    start=(kt == 0), stop=(kt == self.nkt - 1))
        deferred.append(self.norm)

    def norm(self):
        env, ch = self.env, self.ch
        nc, qpool, pc = env['nc'], env['qpool'], env['sc']
        ctx_ps = self.ctx_ps
        rec = pc.tile([1, 512], BF16, tag='rec', bufs=2, name='rec')
        with nc.allow_low_precision(reason='softmax denom recip, ~0.4%'):
            nc.vector.reciprocal(rec[:], ctx_ps[64:65, :])
        bc_sb = pc.tile([64, 512], BF16, tag='bc_sb', bufs=2, name='bc_sb')
        if self.use_pb:
            # GPSIMD broadcast — only while no collective occupies Pool
            nc.gpsimd.partition_broadcast(bc_sb[:], rec[:])
        else:
            bc = qpool.tile([128, 512], F32, tag='proj', bufs=2, name='bc')
            nc.tensor.matmul(bc[0:64, :], env['ones_bf'][0:1, 0:64], rec[:],
                             start=True, stop=True)
            nc.vector.tensor_copy(bc_sb[:], bc[0:64, :])
        ctx_sb = pc.tile([64, 512], BF16, tag='ctx_sb', bufs=2, name='ctx_sb')
        nc.vector.tensor_mul(ctx_sb[:], ctx_ps[0:64, :], bc_sb[:])
        nc.sync.dma_start(self.a2a_in[ch, :, :], ctx_sb[:])


def _build(phases='LE'):
    key = ('nc', phases)
    if key in _cache:
        return _cache[key]
    nc = bacc.Bacc('TRN2', target_bir_lowering=False, debug=False, num_devices=NC)

    xt_d = nc.dram_tensor('xt', [H, T], BF16, kind='ExternalInput')
    wqt_d = nc.dram_tensor('wqt', [H, 128], BF16, kind='ExternalInput')
    wkt_d = nc.dram_tensor('wkt', [H, 128], BF16, kind='ExternalInput')
    wvt_d = nc.dram_tensor('wvt', [H, 128], BF16, kind='ExternalInput')
    wot_d = nc.dram_tensor('wot', [H, H], BF16, kind='ExternalInput')
    bq_d = nc.dram_tensor('bq', [128, 1], F32, kind='ExternalInput')
    bk_d = nc.dram_tensor('bk', [128, 1], F32, kind='ExternalInput')
    bv_d = nc.dram_tensor('bv', [1, 128], F32, kind='ExternalInput')
    bo_d = nc.dram_tensor('bo', [128, HT], F32, kind='ExternalInput')
    out_d = nc.dram_tensor('out', [H, TC], BF16, kind='ExternalOutput')

    with TileContext(nc) as tc:
        with tc.tile_pool(name='persist', bufs=1) as pp, \
             tc.tile_pool(name='scr', bufs=1) as sc, \
             tc.tile_pool(name='dram', bufs=1, space='DRAM') as dpool, \
             tc.tile_pool(name='psum', bufs=1, space='PSUM') as qpool:

            def ptile(shape, dt, tag):
                return pp.tile(shape, dt, tag=tag, name=tag)

            ident = ptile([128, 128], BF16, 'ident')
            make_identity(nc, ident[:])
            ltm = ptile([128, 128], BF16, 'ltm')
            make_lower_triangular(nc, ltm[:], val=NEG, diag=False)
            ones_bf = ptile([1, 128], BF16, 'ones_bf')
            nc.vector.memset(ones_bf[:], 1.0)

            bq_r = ptile([128, 1], F32, 'bq_r')
            bk_r = ptile([128, 1], F32, 'bk_r')
            bo_r = ptile([128, HT], F32, 'bo_r')
            bv_f = ptile([1, 128], F32, 'bv_f')
            bv_bf = ptile([1, 128], BF16, 'bv_bf')

            wqT = ptile([128, HT * 128], BF16, 'wqT')
            wkT = ptile([128, HT * 128], BF16, 'wkT')
            wvT = ptile([128, HT * 128], BF16, 'wvT')
            woT = ptile([128, HT * H], BF16, 'woT')
            xT = ptile([128, HT * T], BF16, 'xT')
            qT = ptile([128, T], BF16, 'qT')
            kT = ptile([128, T], BF16, 'kT')
            v1 = ptile([128, NTT * 130], BF16, 'v1')
            a2a_in0 = dpool.tile([NCHUNK, 64, TC], BF16)
            a2a_out0 = dpool.tile([NCHUNK, 64, TC], BF16)
            a2a_in1 = dpool.tile([NCHUNK, 64, TC], BF16)
            a2a_out1 = dpool.tile([NCHUNK, 64, TC], BF16)

            # ---- input loads, in consumption order (DMA device is serial):
            # biases (tiny), x chunk 0 per-h-tile (so QKV(0) streams), q/k/v
            # weights, remaining x chunks. woT/bo only feed E — loaded
            # mid-L1 (emitted after X0's collective).
            xt_r = xt_d[:].rearrange('(ht p) t -> p ht t', p=128)
            xT_r = xT[:].rearrange('p (ht t) -> p ht t', ht=HT)

            nc.sync.dma_start(
                wqT[:].rearrange('p (ht c) -> p ht c', ht=HT),
                wqt_d[:].rearrange('(ht p) c -> p ht c', p=128))
            for ht in range(HT):
                nc.sync.dma_start(xT_r[:, ht, 0:TC], xt_r[:, ht, 0:TC])
            for w_sb, w_d in ((wkT, wkt_d), (wvT, wvt_d)):
                nc.sync.dma_start(
                    w_sb[:].rearrange('p (ht c) -> p ht c', ht=HT),
                    w_d[:].rearrange('(ht p) c -> p ht c', p=128))
            for dst, src in ((bq_r, bq_d), (bk_r, bk_d), (bv_f, bv_d)):
                nc.sync.dma_start(dst[:], src[:])
            nc.vector.tensor_copy(bv_bf[:], bv_f[:])
            for ch in range(1, NCHUNK):
                nc.sync.dma_start(xT_r[:, :, TC * ch:TC * (ch + 1)],
                                  xt_r[:, :, TC * ch:TC * (ch + 1)])

            # v1 ones columns (col 64 of each 65-block)
            ones_dst = bass.AP(v1.tensor, v1.offset + 64,
                               [list(v1.ap[0]), [130, NTT], [65, 2]])
            nc.vector.memset(ones_dst, 1.0)

            deferred = []
            env = dict(nc=nc, qpool=qpool, sc=sc, qT=qT, kT=kT, v1=v1,
                       ident=ident, ltm=ltm, ones_bf=ones_bf,
                       deferred=deferred)

            def qkv(ch):
                for w_t, b_t, dst in ((wqT, bq_r, qT), (wkT, bk_r, kT)):
                    ps = qpool.tile([128, 512], F32, tag='proj', bufs=2,
                                    name='proj')
                    for ht in range(HT):
                        nc.tensor.matmul(
                            ps[:], w_t[:, 128 * ht:128 * (ht + 1)],
                            xT_r[:, ht, TC * ch:TC * (ch + 1)],
                            start=(ht == 0), stop=(ht == HT - 1))
                    nc.vector.tensor_scalar_add(
                        dst[:, TC * ch:TC * (ch + 1)], ps[:], b_t[:])
                # V in token-major: out[t, c], 4 regions of one bank
                ps = qpool.tile([128, 512], F32, tag='proj', bufs=2,
                                name='proj')
                for tt in range(4):
                    r = ps[:, 128 * tt:128 * (tt + 1)]
                    for ht in range(HT):
                        nc.tensor.matmul(
                            r,
                            xT_r[:, ht,
                                 TC * ch + 128 * tt:TC * ch + 128 * (tt + 1)],
                            wvT[:, 128 * ht:128 * (ht + 1)],
                            start=(ht == 0), stop=False)
                    nc.tensor.matmul(r, ones_bf[:], bv_bf[:],
                                     start=False, stop=True)
                    kt = 4 * ch + tt
                    base = 130 * kt
                    # [V_h0 | gap | V_h1]: strided copy fills cols
                    # base..base+63 and base+65..base+128
                    dst = bass.AP(v1.tensor, v1.offset + base,
                                  [list(v1.ap[0]), [65, 2], [1, 64]])
                    nc.vector.tensor_copy(
                        dst,
                        ps[:, 128 * tt:128 * (tt + 1)].rearrange(
                            'p (g c) -> p g c', g=2))

            ctxa = []
            if 'L' in phases:
                l1 = [_Att(env, ch, 0, a2a_in0, True) for ch in range(NCHUNK)]
                l2 = [_Att(env, ch, 1, a2a_in1, False) for ch in range(NCHUNK)]
                # ---- L1: per-chunk QKV + head-0 attention ----
                for ch in range(NCHUNK):
                    qkv(ch)
                    # next chunk's QKV is the natural PE filler here; a
                    # starter would read qT/kT before they're written
                    l1[ch].body(l2[0] if ch + 1 == NCHUNK else None)
                for fn in deferred:
                    fn()
                deferred.clear()
                # ---- X0: AllToAll for head 0 (overlaps L2) ----
                nc.gpsimd.collective_compute(
                    'AllToAll', mybir.AluOpType.bypass,
                    replica_groups=[list(range(NC))],
                    ins=[a2a_in0[:]], outs=[a2a_out0[:]],
                )
                # E-phase weights + head-0 ctx (lands mid-L2): load now so
                # E_a can fill the AllToAll#1 window (SP queue is in-order)
                nc.sync.dma_start(
                    woT[:].rearrange('p (ct o) -> p ct o', ct=HT),
                    wot_d[:].rearrange('(ct p) o -> p ct o', p=128))
                nc.sync.dma_start(bo_r[:], bo_d[:])
                for jj in range(4):
                    t = ptile([128, 512], BF16, f'ctxa0{jj}')
                    nc.sync.dma_start(
                        t[:],
                        a2a_out0[2 * jj:2 * jj + 2, :, :].rearrange(
                            'a b t -> (a b) t'))
                    ctxa.append(t)

                # ---- L2: head-1 attention ----
                for ch in range(NCHUNK):
                    l2[ch].body(l2[ch + 1] if ch + 1 < NCHUNK else None)
                for fn in deferred:
                    fn()
                deferred.clear()
                nc.gpsimd.collective_compute(
                    'AllToAll', mybir.AluOpType.bypass,
                    replica_groups=[list(range(NC))],
                    ins=[a2a_in1[:]], outs=[a2a_out1[:]],
                )

            # ---- E: transposed output projection for my 512 tokens ----
            # ct-major: E_a (head-0 contraction half) runs during AllToAll#1
            # in 8 held PSUM banks; E_b consumes each ctxa1 tile as it lands;
            # the final ct walks ot-major so copies/stores pipeline out.
            if 'E' in phases:
                slots = []
                for _ in range(2):
                    tl = qpool.tile([128, 1024], F32, tag='stp', bufs=2,
                                    name='stp')
                    slots += [(tl, 0), (tl, 512)]
                for tag in ('proj', 'proj', 'ctx', 'ctx'):
                    slots.append(
                        (qpool.tile([128, 512], F32, tag=tag, bufs=2,
                                    name=tag), 0))

                def e_pass(ct, start, stop, drain=False):
                    for ot in range(HT):
                        tl, c0 = slots[ot]
                        reg = tl[:, c0:c0 + 512]
                        nc.tensor.matmul(
                            reg,
                            woT[:, H * ct + 128 * ot:H * ct + 128 * (ot + 1)],
                            ctxa[ct][:],
                            start=start, stop=stop)
                        if drain:
                            o_sb = sc.tile([128, 512], BF16, tag='o_sb',
                                           bufs=4, name='o_sb')
                            if ot % 2 == 0:
                                nc.vector.tensor_scalar_add(
                                    o_sb[:], reg, bo_r[:, ot:ot + 1])
                            else:
                                nc.scalar.activation(
                                    o_sb[:], reg, IDENT,
                                    bias=bo_r[:, ot:ot + 1])
                            nc.sync.dma_start(
                                out_d[128 * ot:128 * (ot + 1), :], o_sb[:])

                for ct in range(4):
                    e_pass(ct, ct == 0, False)
                for jj in range(4):
                    t = ptile([128, 512], BF16, f'ctxa1{jj}')
                    nc.sync.dma_start(
                        t[:],
                        a2a_out1[2 * jj:2 * jj + 2, :, :].rearrange(
                            'a b t -> (a b) t'))
                    ctxa.append(t)
                for ct in range(4, HT - 1):
                    e_pass(ct, False, False)
                e_pass(HT - 1, False, True, drain=True)

    nc.compile()
    _cache[key] = nc
    return nc


def kernel(hidden_states, Wq, bq, Wk, bk, Wv, bv, Wo, bo, **run_kwargs):
    import ml_dtypes
    bf16 = ml_dtypes.bfloat16
    nc = _build()
    hs = np.asarray(hidden_states, np.float32).reshape(T, H)
    xT = np.ascontiguousarray(hs.T.astype(bf16))
    Wq, Wk, Wv, Wo = (np.asarray(w, np.float32) for w in (Wq, Wk, Wv, Wo))
    bq, bk, bv, bo = (np.asarray(b, np.float32) for b in (bq, bk, bv, bo))
    # woT rows permuted to AllToAll arrival order: [p0h0, p1h0, .., p7h0,
    # p0h1, ..], where peer p's head h covers channels 128p+64h .. +64.
    perm = np.concatenate([np.arange(128 * p + 64 * h, 128 * p + 64 * h + 64)
                           for h in range(2) for p in range(NC)])
    woT = np.ascontiguousarray(Wo.T[perm].astype(bf16))
    bo_m = np.ascontiguousarray(bo.reshape(HT, 128).T)
    in_maps = []
    for c in range(NC):
        r = slice(128 * c, 128 * (c + 1))
        in_maps.append({
            'xt': xT,
            'wqt': np.ascontiguousarray(Wq[r].T.astype(bf16)),
            'wkt': np.ascontiguousarray(Wk[r].T.astype(bf16)),
            'wvt': np.ascontiguousarray(Wv[r].T.astype(bf16)),
            'wot': woT,
            'bq': np.ascontiguousarray(bq[r].reshape(128, 1)),
            'bk': np.ascontiguousarray(bk[r].reshape(128, 1)),
            'bv': np.ascontiguousarray(bv[r].reshape(1, 128)),
            'bo': bo_m,
        })
    res = run_bass_kernel_spmd(nc, in_maps, core_ids=list(range(NC)), **run_kwargs)
    out = np.empty((T, H), np.float32)
    for c in range(NC):
        out[TC * c:TC * (c + 1), :] = res.results[c]['out'].astype(np.float32).T
    kernel.last_results = res
    return out.reshape(B, S, H)
